# revision 55
# baseline (speedup 1.0000x reference)
"""Trainium2 Bass kernel for nn_MultiHeadCulturalAttention.

Sharding (8 cores, SPMD single program with a partition-id branch):
  cores 0-3: "regular" branch — (batch b = core//2), 3 heads of hd=128 each
  cores 4-7: "cultural" branch — (batch b = (core-4)//2), 1 head of hd=384

Every core computes Q/K/V projections for its 384 feature columns, its
attention maps (fully transposed dataflow: Q^T/K^T produced directly by
weight-stationary projections; scores^T computed per s-tile so the
attention-mask add and softmax scale fold into the Exp activation bias;
AV with V-stationary matmuls produces out^T which feeds the folded
output projection wfold = branch_out_w @ out_w_half), then DMAs a
[2048, 768] partial of the final output. The host sums 4 partials per
batch and adds a precomputed constant bias vector.

Softmax normalization: exp tiles are pair-summed on the Vector engine,
the pair sums reduced over partitions by a ones-matmul into PSUM, and
the reciprocal broadcast via GpSimd; the normalize multiply is deferred
into the next attention tile's slack so the PE never waits on it.

All matmul operands are float32r (full-rate on the PE at N>=256);
accumulation stays fp32 in PSUM.
"""
import ml_dtypes
import numpy as np

import concourse.mybir as mybir
from concourse import bacc
from concourse.tile import TileContext
from concourse.bass_utils import run_bass_kernel_spmd

F32 = mybir.dt.float32
F32R = mybir.dt.float32r
BF16 = mybir.dt.bfloat16
AF = mybir.ActivationFunctionType
ALU = mybir.AluOpType

B, T, E = 2, 2048, 768
NE = E // 128            # 6 e-chunks
NT = T // 128            # 16 tiles along seq
NPAIR = NT // 2
F = 384                  # per-core projection width (3 reg heads / 1 cul head)
NF = F // 128            # 3 f-tiles
SCALE_REG = float(128 ** -0.5)
SCALE_CUL = float(384 ** -0.5)

_NC_CACHE = None


def _proj_q4(nc, pool_ps, pool_out, sb_x, sb_w, sb_bias, tag):
    """Like _proj_transposed but with [128,512] psum quarters so 8 psum
    groups stay open while the x chunks stream in."""
    outs = []
    for f in range(NF):
        sb_o = pool_out.tile([128, T], F32R, tag=f"{tag}{f}", name="sb_o")
        for q4 in range(4):
            ps = pool_ps.tile([128, 512], F32, tag="pq", name="ps_q4")
            for e in range(NE):
                nc.tensor.matmul(
                    ps[:],
                    lhsT=sb_w[:, e * F + f * 128: e * F + (f + 1) * 128],
                    rhs=sb_x[e][:, q4 * 512:(q4 + 1) * 512],
                    start=(e == 0), stop=(e == NE - 1))
            nc.scalar.activation(sb_o[:, q4 * 512:(q4 + 1) * 512], ps[:],
                                 AF.Identity, bias=sb_bias[:, f:f + 1], scale=1.0)
        outs.append(sb_o)
    return outs


def _proj_transposed(nc, pool_ps, pool_out, sb_x, sb_w, sb_bias, tag):
    """Q^T/K^T: out[f,t] = sum_e w[e,f] x^T[e,t], evacuated with per-partition
    bias add. Returns 3 SBUF tiles [128, T] (f32r), one per f-tile."""
    outs = []
    for f in range(NF):
        tiles_th = []
        for th in range(2):  # halves of T -> psum [128, 1024]
            ps = pool_ps.tile([128, 1024], F32, tag="pp")
            for e in range(NE):
                for tq in range(2):
                    nc.tensor.matmul(
                        ps[:, tq * 512:(tq + 1) * 512],
                        lhsT=sb_w[:, e * F + f * 128: e * F + (f + 1) * 128],
                        rhs=sb_x[e][:, th * 1024 + tq * 512: th * 1024 + (tq + 1) * 512],
                        start=(e == 0), stop=(e == NE - 1))
            tiles_th.append(ps)
        sb_o = pool_out.tile([128, T], F32R, tag=f"{tag}{f}")
        for th in range(2):
            nc.scalar.activation(sb_o[:, th * 1024:(th + 1) * 1024], tiles_th[th][:],
                                 AF.Identity, bias=sb_bias[:, f:f + 1], scale=1.0)
        outs.append(sb_o)
    return outs


def _proj_v(nc, pool_ps, pool_out, sb_x, sb_wv):
    """V: out[s,d] = sum_e x^T[e,s] wv[e,d]. Returns 16 tiles [128, F] f32r."""
    outs = []
    for s in range(NT):
        ps = pool_ps.tile([128, F], F32, tag="ppv")
        for e in range(NE):
            nc.tensor.matmul(
                ps[:], lhsT=sb_x[e][:, s * 128: (s + 1) * 128],
                rhs=sb_wv[:, e * F:(e + 1) * F],
                start=(e == 0), stop=(e == NE - 1))
        sb_v = pool_out.tile([128, F], F32R, tag=f"v{s}")
        nc.vector.tensor_copy(sb_v[:], ps[:])
        outs.append(sb_v)
    return outs


def _attention(nc, tc, sb_q, sb_k, sb_v, sb_attn, sb_ones, outT,
               maps, t_win, scale, cmask=None, psc_bufs=2, sc_prio=12,
               quads=False):
    """maps: list of (score_chunk_list, av_list) where av_list is
    [(v_col_chunk, out_tile_idx), ...]. t_win: t window width."""
    n_tq = t_win // 512
    n_av = max(len(avs) for _, avs in maps)
    from contextlib import ExitStack
    stk = ExitStack()
    with stk:
        pools = {
            "tc": tc,
            "sc_prio": sc_prio,
            "quads": quads,
            "psc": stk.enter_context(tc.tile_pool(name="psc", bufs=psc_bufs, space="PSUM")),
            "po": stk.enter_context(tc.tile_pool(name="po", bufs=1, space="PSUM")),
            "psum_sum": stk.enter_context(tc.tile_pool(name="psum_sum", bufs=1, space="PSUM")),
            "wt": stk.enter_context(tc.tile_pool(name="wt", bufs=7)),
            "pair": stk.enter_context(tc.tile_pool(name="pair", bufs=4)),
            "quad": stk.enter_context(tc.tile_pool(name="quad", bufs=3)),
            "cm": stk.enter_context(tc.tile_pool(name="cm", bufs=5)),
            "raw": stk.enter_context(tc.tile_pool(name="raw", bufs=3)),
            "srow": stk.enter_context(tc.tile_pool(name="srow", bufs=3)),
        }
        deferred = []
        tiles = [(chunks, avs, ti) for chunks, avs in maps
                 for ti in range(T // t_win)]
        pre = None
        for i, (chunks, avs, ti) in enumerate(tiles):
            nxt = tiles[i + 1] if i + 1 < len(tiles) else None
            pre = _attention_tile(nc, pools, sb_q, sb_k, sb_v, sb_attn, sb_ones,
                                  outT, chunks, avs, ti, t_win, n_tq, scale,
                                  cmask, deferred, pre, nxt)
        for fn in deferred:
            fn()


def _emit_sc(nc, pools, sb_k, sb_q, chunks, ti, t_win, n_tq, s):
    t0 = ti * t_win
    tc = pools["tc"]
    ps_sc = pools["psc"].tile([128, t_win], F32, tag="psc", name="ps_sc")
    with tc.high_priority(offset=pools["sc_prio"]):
        for ci, c in enumerate(chunks):
            for tq in range(n_tq):
                nc.tensor.matmul(
                    ps_sc[:, tq * 512:(tq + 1) * 512],
                    lhsT=sb_k[c][:, s * 128:(s + 1) * 128],
                    rhs=sb_q[c][:, t0 + tq * 512: t0 + (tq + 1) * 512],
                    start=(ci == 0), stop=(ci == len(chunks) - 1))
    return ps_sc


def _emit_cm_dma(nc, pools, cmask, ti, t_win, cm_tiles, s):
    cm = pools["cm"].tile([128, t_win], F32, tag="cm", name="cm")
    nc.sync.dma_start(out=cm[:], in_=cmask[s * 128:(s + 1) * 128,
                                           ti * t_win:(ti + 1) * t_win])
    cm_tiles[s] = cm


def _attention_tile(nc, pools, sb_q, sb_k, sb_v, sb_attn, sb_ones, outT,
                    chunks, avs, ti, t_win, n_tq, scale, cmask, deferred,
                    pre, next_tile):
    t0 = ti * t_win
    ps_o = [pools["po"].tile([128, t_win], F32, tag=f"po{j}", name=f"ps_o{j}")
            for j, _ in enumerate(avs)]
    ps_sum = pools["psum_sum"].tile([1, t_win], F32, tag="psum_sum", name="ps_sum")

    ngroups = NPAIR // 2 if pools["quads"] else NPAIR

    def emit_ones(k, pair_tiles):
        pb = pair_tiles.pop(k)
        for tq in range(n_tq):
            nc.tensor.matmul(
                ps_sum[0:1, tq * 512:(tq + 1) * 512],
                lhsT=sb_ones[:], rhs=pb[:, tq * 512:(tq + 1) * 512],
                start=(k == 0), stop=(k == ngroups - 1))

    if pre is not None:
        sc_next, cm_tiles = pre
    else:
        cm_tiles = {}
        if cmask is not None:
            _emit_cm_dma(nc, pools, cmask, ti, t_win, cm_tiles, 0)
            _emit_cm_dma(nc, pools, cmask, ti, t_win, cm_tiles, 1)
        sc_next = _emit_sc(nc, pools, sb_k, sb_q, chunks, ti, t_win, n_tq, 0)
    nxt_pre = None
    wts = {}
    pair_raw = {}
    pair_tiles = {}
    for s in range(NT):
        ps_sc = sc_next
        if s + 1 < NT:
            sc_next = _emit_sc(nc, pools, sb_k, sb_q, chunks, ti, t_win,
                               n_tq, s + 1)    # PE: scores one step ahead
        elif next_tile is not None:
            # cross-tile pipelining: next tile's first scores + mask tiles
            # go ahead of this tile's sum/normalize tail.
            nchunks, _navs, nti = next_tile
            ncm = {}
            if cmask is not None:
                _emit_cm_dma(nc, pools, cmask, nti, t_win, ncm, 0)
                _emit_cm_dma(nc, pools, cmask, nti, t_win, ncm, 1)
            nxt_pre = (_emit_sc(nc, pools, sb_k, sb_q, nchunks, nti, t_win,
                                n_tq, 0), ncm)
        if cmask is not None:
            if s + 2 < NT:
                _emit_cm_dma(nc, pools, cmask, ti, t_win, cm_tiles, s + 2)
            nc.vector.tensor_tensor(ps_sc[:], ps_sc[:], cm_tiles.pop(s)[:], ALU.add)
        wt = pools["wt"].tile([128, t_win], F32R, tag="wt", name="wt")
        nc.scalar.activation(wt[:], ps_sc[:], AF.Exp,
                             bias=sb_attn[:, s:s + 1], scale=scale)
        wts[s] = wt
        if s % 2 == 1:
            k = s // 2
            pb = pools["pair"].tile([128, t_win], F32R, tag="pair", name="pb")
            nc.vector.tensor_tensor(pb[:], wts.pop(s - 1)[:], wts.pop(s)[:], ALU.add)
            if not pools["quads"]:
                pair_tiles[k] = pb
            else:
                pair_raw[k] = pb
                if k % 2 == 1:
                    qd = pools["quad"].tile([128, t_win], F32R, tag="quad", name="qd")
                    nc.gpsimd.tensor_tensor(qd[:], pair_raw.pop(k - 1)[:],
                                            pair_raw.pop(k)[:], ALU.add)
                    pair_tiles[k // 2] = qd
        if s >= 2 and deferred:
            deferred.pop(0)()
        for j, (vc, _oidx) in enumerate(avs):
            for tq in range(n_tq):
                nc.tensor.matmul(
                    ps_o[j][:, tq * 512:(tq + 1) * 512],
                    lhsT=sb_v[s][:, vc * 128:(vc + 1) * 128],
                    rhs=wt[:, tq * 512:(tq + 1) * 512],
                    start=(s == 0), stop=(s == NT - 1))
        if pools["quads"]:
            if s % 4 == 3 and s // 4 >= 2:
                # after av(s): the ones-matmul fills the PE's wait-for-exp
                # window instead of delaying the next scores matmul.
                emit_ones(s // 4 - 2, pair_tiles)
        elif s % 2 == 1 and s // 2 >= 2:
            emit_ones(s // 2 - 2, pair_tiles)
    emit_ones(ngroups - 2, pair_tiles)
    emit_ones(ngroups - 1, pair_tiles)

    # tail: evacuate attention outputs (frees PSUM fast), reciprocal of the
    # sums, partition-broadcast; the normalize multiply is deferred into the
    # next tile's s-loop slack.
    raw = []
    for j in range(len(avs)):
        r = pools["raw"].tile([128, t_win], F32R, tag=f"raw{j}", name=f"raw{j}")
        nc.vector.tensor_copy(r[:], ps_o[j][:])
        raw.append(r)
        if j == 0:
            rec_row = pools["srow"].tile([1, t_win], F32, tag="recrow", name="rec_row")
            nc.vector.reciprocal(rec_row[:], ps_sum[:])
    rec_b = pools["srow"].tile([128, t_win], F32, tag="recb", name="rec_b")
    nc.gpsimd.partition_broadcast(rec_b[:], rec_row[:])

    for j, (_vc, oidx) in enumerate(avs):
        def mult(j=j, oidx=oidx, raw=raw, rec_b=rec_b, t0=t0):
            nc.vector.tensor_tensor(outT[oidx][:, t0:t0 + t_win],
                                    raw[j][:], rec_b[:], ALU.mult)
        deferred.append(mult)
    return nxt_pre


def _build_nc(branch=None):
    """branch=None: production SPMD kernel with a partition-id If.
    branch='reg'/'cul': single-branch build (for TimelineSim analysis)."""
    nc = bacc.Bacc()
    d_x = [nc.declare_dram_parameter(f"x{e}", [128, T], BF16, isOutput=False)
           for e in range(NE)]
    d_wq = nc.declare_dram_parameter("wq", [128, NE * F], BF16, isOutput=False)
    d_wk = nc.declare_dram_parameter("wk", [128, NE * F], BF16, isOutput=False)
    d_wv = nc.declare_dram_parameter("wv", [128, NE * F], BF16, isOutput=False)
    d_smalls = nc.declare_dram_parameter("smalls", [128, 2 * NF + NT], F32,
                                         isOutput=False)
    d_wfold = nc.declare_dram_parameter("wfold", [128, NF * E], F32R, isOutput=False)
    d_ones = nc.declare_dram_parameter("ones", [128, 1], F32R, isOutput=False)
    d_cmask = nc.declare_dram_parameter("cmask", [T, T], F32, isOutput=False)
    d_out = nc.declare_dram_parameter("out", [T, E], BF16, isOutput=True)

    with TileContext(nc) as tc:
        pid = nc.partition_id()
        from contextlib import ExitStack
        with ExitStack() as stk:
            # ---- persistent pools (live through maps + fold) ----
            p_small = stk.enter_context(tc.tile_pool(name="small", bufs=1))
            p_qt = stk.enter_context(tc.tile_pool(name="qt", bufs=1))
            p_kt = stk.enter_context(tc.tile_pool(name="kt", bufs=1))
            p_v = stk.enter_context(tc.tile_pool(name="vp", bufs=1))
            p_outT = stk.enter_context(tc.tile_pool(name="outT", bufs=1))
            p_wfold = stk.enter_context(tc.tile_pool(name="wfp", bufs=1))

            sb_smalls = p_small.tile([128, 2 * NF + NT], F32)
            sb_ones = p_small.tile([128, 1], F32R)
            sb_dummy = p_small.tile([128, 1], F32)
            nc.sync.dma_start(out=sb_ones[:], in_=d_ones[:])
            sb_qb = sb_smalls[:, 0:NF]
            sb_kb = sb_smalls[:, NF:2 * NF]
            sb_attn = sb_smalls[:, 2 * NF:2 * NF + NT]
            # preload the exp table set while the input DMAs stream
            nc.scalar.activation(sb_dummy[:], sb_ones[:], AF.Exp, bias=0.0,
                                 scale=1.0)
            sb_wfold = p_wfold.tile([128, NF * E], F32R)

            outT = [p_outT.tile([128, T], F32R, tag=f"outT{j}", name=f"outT{j}")
                    for j in range(NF)]

            # ---- projection phase (pools closed afterwards) ----
            with tc.tile_pool(name="xw", bufs=1) as p_xw:
                sb_wq = p_xw.tile([128, NE * F], BF16)
                sb_wk = p_xw.tile([128, NE * F], BF16)
                sb_wv = p_xw.tile([128, NE * F], BF16)
                sb_x = []
                for e in range(NE):
                    t = p_xw.tile([128, T], BF16, tag=f"x{e}", name=f"sb_x{e}")
                    sb_x.append(t)
                nc.sync.dma_start(out=sb_wq[:, 0:F], in_=d_wq[:, 0:F])
                nc.sync.dma_start(out=sb_x[0][:], in_=d_x[0][:])
                nc.sync.dma_start(out=sb_smalls[:], in_=d_smalls[:])
                for e in range(1, NE):
                    nc.sync.dma_start(out=sb_wq[:, e * F:(e + 1) * F],
                                      in_=d_wq[:, e * F:(e + 1) * F])
                    nc.sync.dma_start(out=sb_x[e][:], in_=d_x[e][:])
                nc.sync.dma_start(out=sb_wk[:], in_=d_wk[:])
                nc.sync.dma_start(out=sb_wv[:], in_=d_wv[:])
                nc.sync.dma_start(out=sb_wfold[:], in_=d_wfold[:])
                # Q with [128,512] psum quarters: 8 open accumulation groups
                # keep the PE fed while the x chunks stream in.
                with tc.tile_pool(name="pq", bufs=8, space="PSUM") as p_pq:
                    sb_q = _proj_q4(nc, p_pq, p_qt, sb_x, sb_wq, sb_qb, "q")
                with tc.tile_pool(name="pps", bufs=3, space="PSUM") as p_pps, \
                     tc.tile_pool(name="ppv", bufs=2, space="PSUM") as p_ppv:
                    sb_k = _proj_transposed(nc, p_pps, p_kt, sb_x, sb_wk, sb_kb, "k")
                    sb_v = _proj_v(nc, p_ppv, p_v, sb_x, sb_wv)

            # ---- attention maps (branch on core id) ----
            def _attn_reg():
                _attention(nc, tc, sb_q, sb_k, sb_v, sb_attn,
                           sb_ones, outT,
                           maps=[([m], [(m, m)]) for m in range(3)],
                           t_win=1024, scale=SCALE_REG, sc_prio=24)

            def _attn_cul():
                _attention(nc, tc, sb_q, sb_k, sb_v, sb_attn,
                           sb_ones, outT,
                           maps=[([0, 1, 2], [(0, 0), (1, 1), (2, 2)])],
                           t_win=512, scale=SCALE_CUL, cmask=d_cmask,
                           psc_bufs=4, quads=True)

            if branch == 'reg':
                _attn_reg()
            elif branch == 'cul':
                _attn_cul()
            else:
                with tc.If(pid < 4) as cmp:
                    _attn_reg()
                with cmp.Else():
                    _attn_cul()

            # ---- fold: out[t, :] = sum_c outT[c].T @ wfold[c] ----
            with tc.tile_pool(name="pf", bufs=4, space="PSUM") as p_pf, \
                 tc.tile_pool(name="fin", bufs=6) as p_fin:
                for tt in range(NT):
                    ps_f = p_pf.tile([128, E], F32, tag="pf")
                    for c in range(NF):
                        for e0, e1 in ((0, 512), (512, 768)):
                            nc.tensor.matmul(
                                ps_f[:, e0:e1],
                                lhsT=outT[c][:, tt * 128:(tt + 1) * 128],
                                rhs=sb_wfold[:, c * E + e0: c * E + e1],
                                start=(c == 0), stop=(c == NF - 1))
                    fin = p_fin.tile([128, E], BF16, tag="fin")
                    if tt % 2 == 0:
                        nc.vector.tensor_copy(fin[:], ps_f[:])
                    else:
                        nc.scalar.activation(fin[:], ps_f[:], AF.Identity,
                                             bias=0.0, scale=1.0)
                    nc.sync.dma_start(out=d_out[tt * 128:(tt + 1) * 128, :], in_=fin[:])
    nc.compile()
    return nc


def _get_nc():
    global _NC_CACHE
    if _NC_CACHE is None:
        _NC_CACHE = _build_nc()
    return _NC_CACHE


def _chunked_T(a):
    """[E, T]-style [768, X] -> [128, 6*X] with e-chunk-major free layout."""
    e, x = a.shape
    return np.ascontiguousarray(
        a.reshape(e // 128, 128, x).transpose(1, 0, 2).reshape(128, (e // 128) * x))


def _prepare_in_maps(hidden_states, cultural_mask, attention_mask,
                     rq_w, rk_w, rv_w, ro_w, cq_w, ck_w, cv_w, co_w,
                     rq_b, rk_b, rv_b, ro_b, cq_b, ck_b, cv_b, co_b,
                     r_cb, c_cb, out_w, out_b):
    hidden_states = np.asarray(hidden_states)
    Wo1 = np.asarray(out_w[:E], np.float64)
    Wo2 = np.asarray(out_w[E:], np.float64)
    wfold_reg = (np.asarray(ro_w, np.float64) @ Wo1)
    wfold_cul = (np.asarray(co_w, np.float64) @ Wo2)
    r_cb_flat = np.asarray(r_cb, np.float64).reshape(-1)  # [NH_REG*128] = [768]
    c_cb_flat = np.asarray(c_cb, np.float64).reshape(-1)  # [NH_CUL*384] = [768]
    qb_reg_full = np.asarray(rq_b, np.float64) + r_cb_flat
    qb_cul_full = np.asarray(cq_b, np.float64) + c_cb_flat

    ones = np.ones((128, 1), np.float32)
    zeros_cm = np.zeros((T, T), np.float32)
    in_maps = []
    for core in range(8):
        if core < 4:
            b, h0 = core // 2, (core % 2) * 3
            cols = slice(h0 * 128, h0 * 128 + F)
            wq_l, wk_l, wv_l = rq_w[:, cols], rk_w[:, cols], rv_w[:, cols]
            qb_l = qb_reg_full[cols]
            kb_l = np.asarray(rk_b, np.float64)[cols]
            wfold_l = wfold_reg[cols]
            cm_l = zeros_cm
        else:
            b, h = (core - 4) // 2, (core - 4) % 2
            cols = slice(h * F, (h + 1) * F)
            wq_l, wk_l, wv_l = cq_w[:, cols], ck_w[:, cols], cv_w[:, cols]
            qb_l = qb_cul_full[cols]
            kb_l = np.asarray(ck_b, np.float64)[cols]
            wfold_l = wfold_cul[cols]
            cm_l = np.ascontiguousarray(
                np.asarray(cultural_mask[b], np.float32).T) * np.float32(1.0 / SCALE_CUL)
        xT = np.asarray(hidden_states[b], np.float32).T  # [768, 2048]
        smalls = np.concatenate([
            np.asarray(qb_l, np.float32).reshape(NF, 128).T,
            np.asarray(kb_l, np.float32).reshape(NF, 128).T,
            np.asarray(attention_mask[b, 0, 0, :], np.float32).reshape(NT, 128).T,
        ], axis=1)
        im = {
            "wq": _chunked_T(np.asarray(wq_l, np.float32)).astype(ml_dtypes.bfloat16),
            "wk": _chunked_T(np.asarray(wk_l, np.float32)).astype(ml_dtypes.bfloat16),
            "wv": _chunked_T(np.asarray(wv_l, np.float32)).astype(ml_dtypes.bfloat16),
            "smalls": np.ascontiguousarray(smalls),
            "wfold": _chunked_T(np.asarray(wfold_l, np.float32)),
            "ones": ones,
            "cmask": cm_l,
        }
        xc = np.ascontiguousarray(xT).reshape(NE, 128, T)
        for e in range(NE):
            im[f"x{e}"] = np.ascontiguousarray(xc[e]).astype(ml_dtypes.bfloat16)
        in_maps.append(im)

    bias_total = (np.asarray(out_b, np.float64)
                  + np.asarray(ro_b, np.float64) @ Wo1
                  + np.asarray(co_b, np.float64) @ Wo2
                  + np.asarray(rv_b, np.float64) @ np.asarray(ro_w, np.float64) @ Wo1
                  + np.asarray(cv_b, np.float64) @ np.asarray(co_w, np.float64) @ Wo2)
    return in_maps, bias_total


def kernel(hidden_states, cultural_mask, attention_mask,
           rq_w, rk_w, rv_w, ro_w, cq_w, ck_w, cv_w, co_w,
           rq_b, rk_b, rv_b, ro_b, cq_b, ck_b, cv_b, co_b,
           r_cb, c_cb, out_w, out_b):
    nc = _get_nc()
    in_maps, bias_total = _prepare_in_maps(
        hidden_states, cultural_mask, attention_mask,
        rq_w, rk_w, rv_w, ro_w, cq_w, ck_w, cv_w, co_w,
        rq_b, rk_b, rv_b, ro_b, cq_b, ck_b, cv_b, co_b,
        r_cb, c_cb, out_w, out_b)

    res = run_bass_kernel_spmd(nc, in_maps, list(range(8))).results

    out = np.empty((B, T, E), np.float32)
    for b in range(B):
        acc = (res[2 * b]["out"].astype(np.float64)
               + res[2 * b + 1]["out"].astype(np.float64)
               + res[4 + 2 * b]["out"].astype(np.float64)
               + res[5 + 2 * b]["out"].astype(np.float64)
               + bias_total)
        out[b] = acc.astype(np.float32)
    return out
